# revision 31
# baseline (speedup 1.0000x reference)
"""MoE (base FFN + top-2-of-8 expert FFNs) on 8 TRN2 NeuronCores.

Strategy (uniform weight-slots, balanced):
  - Routing (softmax over 8 experts, top-2, renormalize) is computed on
    host with jax-CPU, mirroring the reference computation exactly.
  - The device program is 3 uniform "jobs" (slots) per core with
    compile-time capacities (s1, s2, s3).  Each slot is a full FFN pass
    over its own token matrix with its OWN weight inputs and a per-token
    scale row.  Base FFN is just another weight set with scale 1.0, so
    base work and expert work share the same slot structure.
  - A tiny host-side search picks (s1, s2, s3) and assigns each expert's
    token list to <=3 slots and the 4096 base tokens to the leftover
    slots, so per-core work is ~1546 token-passes (vs 1536 ideal).
  - Host scatter-adds all slot outputs.

Device compute in bf16 with fp32 PSUM accumulation; activations stay in
[feature, token] layout so both FFN matmuls chain without transposes.
Token/scale loads are issued from the sync engine, weight-tile loads
from the scalar engine, and output stores rotate over gpsimd/sync/
scalar so no DMA stream head-of-line-blocks another.
"""

import numpy as np
import ml_dtypes

import concourse.bass as bass
import concourse.mybir as mybir
import concourse.tile as tile
from concourse import bacc
from concourse.bass_utils import run_bass_kernel_spmd
from concourse.tile_rust import add_dep_helper

P = 128
B, S, H, F, E = 2, 2048, 1024, 4096, 8
T = B * S
TOP_K = 2
BETA = 1.0

F32 = mybir.dt.float32
CDT = mybir.dt.bfloat16  # compute dtype on the tensor engine
NP_CDT = ml_dtypes.bfloat16

KA = H // P   # 8  k-subtiles contracting H
FB = F // P   # 32 output blocks of F
KB = F // P   # 32 k-subtiles contracting F
HB = H // P   # 8  output blocks of H
NSLOT = 3


def _chunks(s):
    """PSUM chunks for a slot of s tokens.  A full 512 chunk plus a small
    remainder beats an even split: the 512-row matmuls leave plenty of
    slack for the ~100ns per-instruction issue work, which a pair of
    ~273-row matmuls does not."""
    if s <= 512:
        return [(0, s)]
    h = ((s + 1) // 2 + 1) // 2 * 2
    return [(0, h), (h, s - h)]


def _stage(nc, wpool, pspool, wtag, w_d, x_s, cts, evict, wt0=None, wengs=None):
    """One matmul stage: out[ob] = evict(sum_k w[ob,k].T @ x[k]) per chunk.

    w_d: DRAM [P, OB, K, 128]; x_s: SBUF [P, K, n_cols].
    wt0: optional pre-loaded weight tile for ob==0.
    wengs: engines to rotate weight-tile dma issues over (default scalar).
    """
    OB, K = w_d.shape[1], w_d.shape[2]
    if wengs is None:
        wengs = [nc.scalar]
    for ob in range(OB):
        if ob == 0 and wt0 is not None:
            wt = wt0
        else:
            wt = wpool.tile([P, K, P], CDT, name=wtag)
            wengs[ob % len(wengs)].dma_start(out=wt[:], in_=w_d[:, ob])
        pss = [
            (pspool.tile([P, 512], F32, name="ps"), c0, cn) for c0, cn in cts
        ]
        for k in range(K):
            for ps, c0, cn in pss:
                nc.tensor.matmul(
                    ps[:, :cn],
                    wt[:, k],
                    x_s[:, k, c0 : c0 + cn],
                    start=(k == 0),
                    stop=(k == K - 1),
                )
        for ps, c0, cn in pss:
            evict(ob, ps, c0, cn)


def _build(sizes):
    """Build the per-core SPMD program for slot capacities `sizes`."""
    nc = bacc.Bacc(None, target_bir_lowering=False, debug=False)
    act_silu = mybir.ActivationFunctionType.Silu
    smax = max(sizes)
    with tile.TileContext(nc) as tc:
        with tc.tile_pool(name="dram", bufs=1, space="DRAM") as dram:
            kw = dict(kind="ExternalInput", uniquify=False)
            xs, w1s, w2s, scs, ys = [], [], [], [], []
            for j, s in enumerate(sizes):
                xs.append(dram.tile((P, KA, s), CDT, name=f"x{j}", **kw))
                w1s.append(dram.tile((P, FB, KA, P), CDT, name=f"w1{j}", **kw))
                w2s.append(dram.tile((P, HB, KB, P), CDT, name=f"w2{j}", **kw))
                scs.append(dram.tile((P, s), F32, name=f"sc{j}", **kw))
                ys.append(
                    dram.tile(
                        (P, HB, s), CDT, name=f"y{j}",
                        kind="ExternalOutput", uniquify=False,
                    )
                )
            with (
                tc.tile_pool(name="res", bufs=1) as res,
                tc.tile_pool(name="hh", bufs=2) as hh,
                tc.tile_pool(name="wa", bufs=12) as wa,
                tc.tile_pool(name="wb", bufs=5) as wb,
                tc.tile_pool(name="ps", bufs=8, space="PSUM") as ps,
                tc.tile_pool(name="yo", bufs=4) as yo,
            ):
                # slot 0 startup loads first: the DMA path ramps up from
                # ~40GB/s cold to >160GB/s warm, so the leading x pieces are
                # small (one k-subtile) to get the first matmuls' operands
                # in as early as possible.
                wt0 = wa.tile([P, KA, P], CDT, name="wa")
                nc.scalar.dma_start(out=wt0[:, 0:1], in_=w1s[0][:, 0, 0:1])
                nc.scalar.dma_start(out=wt0[:, 1:3], in_=w1s[0][:, 0, 1:3])
                nc.scalar.dma_start(out=wt0[:, 3:KA], in_=w1s[0][:, 0, 3:KA])
                x0_s = res.tile([P, KA, sizes[0]], CDT, name="x0_s")
                for k0, k1 in ((0, 2), (2, 5), (5, KA)):
                    nc.sync.dma_start(out=x0_s[:, k0:k1], in_=xs[0][:, k0:k1])

                # PE warm-up: junk matmuls on a zeroed scratch tile while the
                # first real loads are in flight, so the HAM clock gate is
                # already at full rate when real matmuls start.
                wlhs = res.tile([P, P], CDT, name="wlhs")
                nc.gpsimd.memset(wlhs[:], 0.0)
                wrhs = res.tile([P, 512], CDT, name="wrhs")
                nc.gpsimd.memset(wrhs[:], 0.0)
                wps = ps.tile([P, 512], F32, name="ps")
                for _ in range(8):
                    nc.tensor.matmul(wps[:], wlhs[:], wrhs[:], start=True, stop=True)

                out_engines = [nc.gpsimd, nc.sync, nc.scalar]
                ev_n = [0]

                def _out_dma(dst, o, cn, engines=None):
                    engs = engines or out_engines
                    eng = engs[ev_n[0] % len(engs)]
                    ev_n[0] += 1
                    eng.dma_start(out=dst, in_=o[:, :cn])

                x_tiles = [x0_s, None, None]
                for j, s in enumerate(sizes):
                    cts = _chunks(s)
                    x_s = x_tiles[j]
                    h = hh.tile([P, KB, smax], CDT, name="hh")

                    marker = []

                    def ev_l1(ob, psum, c0, cn, h=h, marker=marker):
                        act = nc.scalar.activation(
                            h[:, ob, c0 : c0 + cn], psum[:, :cn], act_silu
                        )
                        if ob == 1:
                            marker.append(act)

                    _stage(
                        nc, wa, ps, "wa", w1s[j], x_s, cts, ev_l1,
                        wt0=wt0 if j == 0 else None,
                    )

                    # next slot's tokens: loaded during this slot's layer-1,
                    # gated on early progress so the transfer never competes
                    # with the startup-critical loads.
                    if j + 1 < NSLOT:
                        xn = res.tile(
                            [P, KA, sizes[j + 1]], CDT, name=f"x{j+1}_s"
                        )
                        for k in range(0, KA, 4):
                            dma = nc.sync.dma_start(
                                out=xn[:, k : k + 4], in_=xs[j + 1][:, k : k + 4]
                            )
                            add_dep_helper(
                                dma.ins,
                                marker[0].ins,
                                reason="defer next-slot x load past startup",
                            )
                        x_tiles[j + 1] = xn

                    sc_s = res.tile([P, s], F32, name=f"sc{j}_s")
                    nc.sync.dma_start(out=sc_s[:], in_=scs[j][:])

                    last = j == NSLOT - 1

                    def ev_l2(ob, psum, c0, cn, y=ys[j], sc=sc_s, last=last):
                        o = yo.tile([P, 512], CDT, name="yo")
                        nc.vector.tensor_tensor(
                            out=o[:, :cn],
                            in0=psum[:, :cn],
                            in1=sc[:, c0 : c0 + cn],
                            op=mybir.AluOpType.mult,
                        )
                        # final stage: fast-issue engines so the tail store
                        # chain after the last matmul is as short as possible
                        _out_dma(
                            y[:, ob, c0 : c0 + cn], o, cn,
                            engines=[nc.sync, nc.scalar] if last else None,
                        )

                    _stage(nc, wb, ps, "wb", w2s[j], h, cts, ev_l2)
    nc.compile()
    return nc


_BUILD_CACHE = {}


def _get_program(sizes):
    if sizes not in _BUILD_CACHE:
        _BUILD_CACHE[sizes] = _build(sizes)
    return _BUILD_CACHE[sizes]


def _plan(counts):
    """Pick slot capacities (s1>=s2>=s3) and an expert->slots assignment.

    Each of the 8 cores runs one slot of each capacity, so there are 8
    physical slots per capacity class.  Every expert's n_e tokens must be
    covered by <=3 slots used exclusively; base (4096 tokens) takes the
    leftover slots.  Returns (sizes, assign) where assign[e] is a list of
    capacity-class indices for expert e (sorted desc by count order).
    """
    order = sorted(range(E), key=lambda e: -counts[e])
    cnts = [counts[e] for e in order]
    nmax = max(cnts) if cnts else 0

    def feasible(sizes):
        usage = [8, 8, 8]
        assign = [None] * len(cnts)

        def options(n):
            opts = []
            for i in range(3):
                if sizes[i] >= n:
                    opts.append((sizes[i], (i,)))
            for i in range(3):
                for jj in range(i, 3):
                    sm = sizes[i] + sizes[jj]
                    if sm >= n:
                        opts.append((sm, (i, jj)))
            for i in range(3):
                for jj in range(i, 3):
                    for k in range(jj, 3):
                        sm = sizes[i] + sizes[jj] + sizes[k]
                        if sm >= n:
                            opts.append((sm, (i, jj, k)))
            opts.sort()
            return [o for _, o in opts]

        def dfs(i):
            avail = sum(usage[k] * sizes[k] for k in range(3))
            if i == len(cnts):
                return avail >= T
            if sum(cnts[i:]) + T > avail:
                return False
            for opt in options(cnts[i]):
                need = {}
                for c in opt:
                    need[c] = need.get(c, 0) + 1
                if all(usage[c] >= m for c, m in need.items()):
                    for c, m in need.items():
                        usage[c] -= m
                    assign[i] = opt
                    if dfs(i + 1):
                        return True
                    for c, m in need.items():
                        usage[c] += m
                    assign[i] = None
            return False

        if dfs(0):
            return list(assign)
        return None

    hi = 2 * ((nmax + 1) // 2) + 1024
    for total in range(1536, hi + 1, 2):
        lo1 = max((total + 2) // 3, total - 1024)
        for s1 in range(lo1, min(1024, total - 256) + 1, 2):
            rem = total - s1
            for s2 in range(min(512, s1, rem - 128), (rem + 1) // 2 - 1, -2):
                s3 = rem - s2
                if s3 < 128 or s3 > s2:
                    continue
                a = feasible((s1, s2, s3))
                if a is not None:
                    assign = {order[i]: a[i] for i in range(len(cnts))}
                    return (s1, s2, s3), assign
    # fallback: always-feasible plan
    s = max(((nmax + 1) // 2 + 15) // 16 * 16, 64)
    sizes = (s, s, 512)
    assign = {order[i]: (0, 1) for i in range(len(cnts))}
    return sizes, assign


def _routing(x, router_w):
    """Replicate the reference router bit-for-bit on jax CPU."""
    import jax
    import jax.numpy as jnp

    cpu = jax.devices("cpu")[0]

    def _route(xj, rj):
        logits = xj @ rj
        probs = jax.nn.softmax(logits, axis=-1)
        top_w, top_i = jax.lax.top_k(probs, TOP_K)
        top_w = top_w / jnp.sum(top_w, axis=-1, keepdims=True)
        return top_w, top_i

    with jax.default_device(cpu):
        top_w, top_i = jax.jit(_route)(jnp.asarray(x), jnp.asarray(router_w))
        top_w = np.asarray(top_w)
        top_i = np.asarray(top_i)
    return top_w, top_i


def _as_pkc(w, kb, nb):
    # [K, N] -> [P, nblocks, kblocks, 128]: w[k*128+p, n*128+c] -> [p, n, k, c]
    return np.ascontiguousarray(w.reshape(kb, P, nb, P).transpose(1, 2, 0, 3))


def _as_pit(xt):
    # [R, N] -> [P, R//128, N]: xt[i*128+p, t] -> [p, i, t]
    r, n = xt.shape
    return np.ascontiguousarray(xt.reshape(r // P, P, n).transpose(1, 0, 2))


def _from_pit(y):
    # [P, R//128, N] -> [N, R]
    p, i, n = y.shape
    return y.transpose(2, 1, 0).reshape(n, i * p)


def kernel(hidden_states, router_w, base_w1, base_w2, exp_w1, exp_w2):
    x = np.ascontiguousarray(hidden_states.reshape(T, H), dtype=np.float32)
    top_w, top_i = _routing(x, np.asarray(router_w, dtype=np.float32))

    # per-expert token lists
    idx = []
    wts = []
    for e in range(E):
        rows, slots = np.nonzero(top_i == e)
        idx.append(rows)
        wts.append(top_w[rows, slots].astype(np.float32))
    counts = [len(r) for r in idx]

    sizes, assign = _plan(counts)
    nc = _get_program(sizes)

    xT_c = np.ascontiguousarray(x.T).astype(NP_CDT)  # [H, T]

    # weight sets in device layout, converted once
    b1_dev = _as_pkc(np.asarray(base_w1, np.float32).astype(NP_CDT), KA, FB)
    b2_dev = _as_pkc(np.asarray(base_w2, np.float32).astype(NP_CDT), KB, HB)
    e1_dev = [
        _as_pkc(np.asarray(exp_w1[e], np.float32).astype(NP_CDT), KA, FB)
        for e in range(E)
    ]
    e2_dev = [
        _as_pkc(np.asarray(exp_w2[e], np.float32).astype(NP_CDT), KB, HB)
        for e in range(E)
    ]

    # physical slots: (core, class) for class j = slot of capacity sizes[j].
    free = [[(c, j) for c in range(E)] for j in range(NSLOT)]
    # slot_fill[core][class] = (tokens, scales, w1_dev, w2_dev)
    slot_fill = [[None] * NSLOT for _ in range(E)]

    for e in sorted(range(E), key=lambda e: -counts[e]):
        pos = 0
        for cls in assign[e]:
            core, j = free[cls].pop()
            take = min(sizes[j], counts[e] - pos)
            sl = slice(pos, pos + take)
            slot_fill[core][j] = (
                idx[e][sl],
                (BETA * wts[e][sl]).astype(np.float32),
                e1_dev[e],
                e2_dev[e],
            )
            pos += take
        assert pos >= counts[e], (e, pos, counts[e])

    base_tokens = np.arange(T)
    pos = 0
    for j in range(NSLOT):
        for core, jj in free[j]:
            take = min(sizes[jj], T - pos)
            sl = base_tokens[pos : pos + take]
            slot_fill[core][jj] = (
                sl,
                np.ones(len(sl), dtype=np.float32),
                b1_dev,
                b2_dev,
            )
            pos += take
    assert pos >= T, pos

    in_maps = []
    slot_tok = [[None] * NSLOT for _ in range(E)]
    for core in range(E):
        m = {}
        for j in range(NSLOT):
            s = sizes[j]
            fill = slot_fill[core][j]
            if fill is None:
                toks = np.empty(0, dtype=np.int64)
                scl = np.empty(0, dtype=np.float32)
                w1d, w2d = b1_dev, b2_dev
            else:
                toks, scl, w1d, w2d = fill
            slot_tok[core][j] = toks
            xg = np.zeros((H, s), dtype=NP_CDT)
            xg[:, : len(toks)] = xT_c[:, toks]
            sc = np.zeros((s,), dtype=np.float32)
            sc[: len(toks)] = scl
            m[f"x{j}"] = _as_pit(xg)
            m[f"sc{j}"] = np.ascontiguousarray(np.broadcast_to(sc, (P, s)))
            m[f"w1{j}"] = w1d
            m[f"w2{j}"] = w2d
        in_maps.append(m)

    res = run_bass_kernel_spmd(nc, in_maps, core_ids=list(range(8)))

    out = np.zeros((T, H), dtype=np.float32)
    for core in range(E):
        for j in range(NSLOT):
            toks = slot_tok[core][j]
            if len(toks) == 0:
                continue
            ym = _from_pit(res.results[core][f"y{j}"])[: len(toks)]
            out[toks] += ym.astype(np.float32)
    return out.reshape(B, S, H)
